# revision 7
# baseline (speedup 1.0000x reference)
"""MAGNN intra-metapath aggregator on 8 TRN2 NeuronCores.

fp16 streaming design. The kernel is HBM-bandwidth bound (the 819 MB
paths tensor must be streamed once; per-NC HBM share is ~358 GB/s), so
the host casts paths to fp16 before upload — 51.2 MB/core instead of
102.4 MB/core, halving the DMA floor to ~143 us. Output error from the
cast is ~2e-5, far under the 2e-2 gate.

Per 2 MB chunk (512 instances, 4 per SBUF partition as 4 KB free-dim
blocks): DVE folds 16 path nodes to 4-node partials in two fp16 adds
(2x_1P mode); PE transposes the partials via fp16 matmuls against an
fp16 identity (fp32 PSUM accumulate) to get repsum^T for the scores;
LeakyReLU+exp on DVE/ACT; PE accumulates alpha-weighted partials and
weight sums into persistent PSUM. Per-core partials [H, D+1] are
combined on the host (cheaper than a device AllReduce).

Tail (12500 = 24*512 + 212) uses a 1-instance-per-partition path."""

import numpy as np

from concourse import bacc, masks, mybir, tile
from concourse.bass_utils import run_bass_kernel_spmd

N, L, D, H = 100000, 16, 128, 8
NCORES = 8
NS = N // NCORES            # 12500 instances per core
CHUNK = 128                 # instances per small (tail) tile
BIG = 512                   # instances per big tile (4 per partition)
NB = 4                      # blocks per partition in a big tile
F32 = mybir.dt.float32
F16 = mybir.dt.float16
PATHS_NP_DTYPE = np.float16
AF = mybir.ActivationFunctionType

_cached_nc = None


def _build(ns=NS, repeat=1, **_compat):
    nbig = ns // BIG
    tail_chunks = []
    t0 = nbig * BIG
    rem = ns - t0
    while rem > 0:
        cnt = min(CHUNK, rem)
        tail_chunks.append((t0, cnt))
        t0 += cnt
        rem -= cnt
    nc = bacc.Bacc(
        "TRN2",
        target_bir_lowering=False,
        debug=False,
        enable_asserts=False,
        num_devices=NCORES,
    )
    paths_d = nc.dram_tensor("paths", [ns, L, D], F16, kind="ExternalInput")
    tgt_d = nc.dram_tensor("target_feat", [D], F32, kind="ExternalInput")
    af_d = nc.dram_tensor("attn_fc", [H, 2 * D], F32, kind="ExternalInput")
    out_d = nc.dram_tensor("out", [H * (D + 1)], F32, kind="ExternalOutput")

    FD = L * D  # 2048 elements per instance

    with tile.TileContext(nc) as tc:
        with (
            tc.tile_pool(name="const", bufs=1) as constp,
            tc.tile_pool(name="inp", bufs=5) as inp,
            tc.tile_pool(name="work", bufs=3) as work,
            tc.tile_pool(name="ps", bufs=1, space="PSUM") as psp,
        ):
            # ---------- constants ----------
            ident = constp.tile([128, 128], F32)
            masks.make_identity(nc, ident[:])
            ident16 = constp.tile([128, 128], F16)
            nc.vector.tensor_copy(ident16[:], ident[:])
            af = constp.tile([H, 2 * D], F32)
            nc.sync.dma_start(af[:], af_d.ap())
            tf = constp.tile([D, 1], F32)
            nc.sync.dma_start(tf[:], tgt_d.ap().rearrange("(d one) -> d one", one=1))
            ones_row = constp.tile([1, CHUNK], F32)
            nc.vector.memset(ones_row[:], 1.0)
            ones_col = constp.tile([CHUNK, 1], F16)
            nc.vector.memset(ones_col[:], 1.0)

            # a_tT [D, H]: transpose of attn_fc[:, :D] (f32, setup only)
            ps_t = psp.tile([128, 128], F32, tag="setup")
            a_tT = constp.tile([D, H], F32)
            nc.tensor.transpose(ps_t[:D, :H], af[:H, 0:D], ident[:H, :H])
            nc.vector.tensor_copy(a_tT[:], ps_t[:D, :H])
            # a_rT [D, H] scaled by 1/16 (folds the path-mean into scores),
            # cast to fp16 so it can pair with the fp16 repsum^T stationary
            ps_r = psp.tile([128, 128], F32, tag="setup")
            a_rT = constp.tile([D, H], F16)
            nc.tensor.transpose(ps_r[:D, :H], af[:H, D : 2 * D], ident[:H, :H])
            nc.scalar.mul(a_rT[:], ps_r[:D, :H], 1.0 / L)
            # per-head bias b[h] = a_t[h] . target  -> kept as a [1, H] row
            ps_b = psp.tile([128, 128], F32, tag="setup")
            b_row = constp.tile([1, H], F32)
            nc.tensor.matmul(ps_b[:1, :H], tf[:, :1], a_tT[:, :H])
            nc.vector.tensor_copy(b_row[:], ps_b[:1, :H])

            # ---------- persistent accumulators ----------
            acc_p = psp.tile([H, 4 * D], F32, tag="accP")  # 4-node partials
            acc_s = psp.tile([H, 1], F32, tag="accS")      # sum_n w[n,h]

            paths2d = paths_d.ap().rearrange("n l d -> n (l d)")

            # ---------- main streaming loop ----------
            started = [False]

            def score_block(tmp_t, off, cnt, e_dst, rT_blk):
                # repsum^T [D, cnt] from the 4 partial-sum d-blocks: fp16
                # matmuls against fp16 identity, fp32 PSUM accumulation.
                pt = psp.tile([128, CHUNK], F32, tag="pt", bufs=2)
                for j in range(4):
                    nc.tensor.matmul(
                        pt[:D, :cnt],
                        tmp_t[:cnt, off + j * D : off + (j + 1) * D],
                        ident16[:cnt, :cnt],
                        start=(j == 0), stop=(j == 3),
                        skip_group_check=True,
                    )
                nc.scalar.copy(rT_blk[:, :cnt], pt[:D, :cnt])
                nc.tensor.matmul(
                    e_dst, ones_row[:1, :cnt], b_row[:1, :],
                    start=True, stop=False, skip_group_check=True,
                )
                nc.tensor.matmul(
                    e_dst, rT_blk[:, :cnt], a_rT[:, :],
                    start=False, stop=True, skip_group_check=True,
                )

            def weights_from_scores(e_ps, wT, cnt, w):
                # LeakyReLU(0.2) then exp; wT in fp16 for the acc matmuls
                sc = work.tile([128, NB * H], F32, tag="sc")
                nc.vector.tensor_scalar_mul(sc[:cnt, :w], e_ps[:cnt, :w], 0.2)
                lr = work.tile([128, NB * H], F32, tag="lr")
                nc.vector.tensor_max(lr[:cnt, :w], sc[:cnt, :w], e_ps[:cnt, :w])
                nc.scalar.activation(wT[:cnt, :w], lr[:cnt, :w], AF.Exp)

            def do_small(n0, cnt, last):
                first = not started[0]
                started[0] = True
                t = inp.tile([128, FD], F16, tag="in", padded_shape=[128, NB * FD])
                nc.sync.dma_start(t[:cnt, :], paths2d[n0 : n0 + cnt, :])
                tmp = work.tile(
                    [128, 1024], F16, tag="tree", padded_shape=[128, NB * 1024]
                )
                nc.vector.tensor_add(
                    tmp[:cnt, :], t[:cnt, 0:1024], t[:cnt, 1024:2048]
                )
                nc.vector.tensor_add(
                    tmp[:cnt, 0:512], tmp[:cnt, 0:512], tmp[:cnt, 512:1024]
                )
                rT = work.tile([D, CHUNK], F16, tag="rT", padded_shape=[D, NB * CHUNK])
                e_ps = psp.tile(
                    [128, H], F32, tag="e", bufs=2, padded_shape=[128, NB * H]
                )
                score_block(tmp, 0, cnt, e_ps[:cnt, :], rT)
                wT = work.tile([128, H], F16, tag="wT", padded_shape=[128, NB * H])
                weights_from_scores(e_ps, wT, cnt, H)
                nc.tensor.matmul(
                    acc_p[:H, :], wT[:cnt, :H], tmp[:cnt, 0:512],
                    start=first, stop=last,
                )
                nc.tensor.matmul(
                    acc_s[:H, :], wT[:cnt, :H], ones_col[:cnt, :],
                    start=first, stop=last,
                )

            def do_big(n0, last):
                first = not started[0]
                started[0] = True
                t = inp.tile([128, NB * FD], F16, tag="in")
                t3 = t.rearrange("p (b f) -> p b f", b=NB)
                nc.sync.dma_start(
                    t3[:, :, :],
                    paths2d[n0 : n0 + BIG, :].rearrange("(b p) f -> p b f", b=NB),
                )
                tmp = work.tile([128, NB * 1024], F16, tag="tree")
                tmp3 = tmp.rearrange("p (b x) -> p b x", b=NB)
                nc.vector.tensor_add(
                    tmp3[:, :, :], t3[:, :, 0:1024], t3[:, :, 1024:2048]
                )
                nc.vector.tensor_add(
                    tmp3[:, :, 0:512], tmp3[:, :, 0:512], tmp3[:, :, 512:1024]
                )
                e_ps = psp.tile([128, NB * H], F32, tag="e", bufs=2)
                rT = work.tile([D, NB * CHUNK], F16, tag="rT")
                for b in range(NB):
                    score_block(
                        tmp,
                        b * 1024,
                        CHUNK,
                        e_ps[:, b * H : (b + 1) * H],
                        rT[:, b * CHUNK : (b + 1) * CHUNK],
                    )
                wT = work.tile([128, NB * H], F16, tag="wT")
                weights_from_scores(e_ps, wT, 128, NB * H)
                for b in range(NB):
                    bfirst = first and b == 0
                    blast = last and b == NB - 1
                    nc.tensor.matmul(
                        acc_p[:H, :],
                        wT[:, b * H : (b + 1) * H],
                        tmp[:, b * 1024 : b * 1024 + 512],
                        start=bfirst, stop=blast,
                    )
                    nc.tensor.matmul(
                        acc_s[:H, :],
                        wT[:, b * H : (b + 1) * H],
                        ones_col[:, :],
                        start=bfirst, stop=blast,
                    )

            # repeat>1 is a timing-only mode (re-streams the same shard;
            # output then over-counts, never used for correctness runs)
            for r in range(repeat):
                lr_ = r == repeat - 1
                for c in range(nbig):
                    do_big(
                        c * BIG,
                        lr_ and not tail_chunks and c == nbig - 1,
                    )
                for i, (n0, cnt) in enumerate(tail_chunks):
                    do_small(n0, cnt, lr_ and i == len(tail_chunks) - 1)

            # ---------- emit per-core partial [p_raw | s] ----------
            # Cross-core combine + softmax normalization happens on the host
            # in kernel(): cheaper than a device AllReduce + bounce trips.
            accs = work.tile([H, 4 * D], F32, tag="accs")
            nc.vector.tensor_copy(accs[:H, :], acc_p[:H, :])
            fold = work.tile([H, 2 * D], F32, tag="fold")
            nc.vector.tensor_add(
                fold[:H, :], accs[:H, 0 : 2 * D], accs[:H, 2 * D : 4 * D]
            )
            part = work.tile([H, D + 1], F32, tag="part")
            nc.vector.tensor_add(part[:H, 0:D], fold[:H, 0:D], fold[:H, D : 2 * D])
            nc.vector.tensor_copy(part[:H, D : D + 1], acc_s[:H, :])
            nc.sync.dma_start(
                out_d.ap().rearrange("(h d) -> h d", d=D + 1), part[:]
            )

    nc.compile()
    return nc


def kernel(target_feat, paths, attn_fc, **_unused):
    global _cached_nc
    if _cached_nc is None:
        _cached_nc = _build()
    nc = _cached_nc

    paths = np.ascontiguousarray(
        np.asarray(paths, dtype=np.float32).astype(PATHS_NP_DTYPE)
    )
    shards = paths.reshape(NCORES, NS, L, D)
    tgt = np.ascontiguousarray(np.asarray(target_feat, dtype=np.float32))
    af = np.ascontiguousarray(np.asarray(attn_fc, dtype=np.float32))
    in_maps = [
        {"paths": shards[i], "target_feat": tgt, "attn_fc": af}
        for i in range(NCORES)
    ]
    res = run_bass_kernel_spmd(nc, in_maps, core_ids=list(range(NCORES)))
    # host-side combine of the 8 per-core partials [8, D+1]
    tot = np.zeros((H, D + 1), dtype=np.float64)
    for i in range(NCORES):
        tot += np.asarray(res.results[i]["out"], dtype=np.float64).reshape(
            H, D + 1
        )
    out = tot[:, :D] / (L * tot[:, D:])
    return np.ascontiguousarray(out.reshape(H * D).astype(np.float32))


# revision 9
# speedup vs baseline: 523.0593x; 523.0593x over previous
"""MAGNN intra-metapath aggregator on 8 TRN2 NeuronCores.

fp16 streaming design. The kernel is HBM-bandwidth bound (the 819 MB
paths tensor must be streamed once; per-NC HBM share is ~358 GB/s), so
the host casts paths to fp16 before upload — 51.2 MB/core instead of
102.4 MB/core, halving the DMA floor to ~143 us. Output error from the
cast is ~2e-5, far under the 2e-2 gate.

Per 2 MB chunk (512 instances, 4 per SBUF partition as 4 KB free-dim
blocks): DVE folds 16 path nodes to 4-node partials in two fp16 adds
(2x_1P mode); PE transposes the partials via fp16 matmuls against an
fp16 identity (fp32 PSUM accumulate) to get repsum^T for the scores;
LeakyReLU+exp on DVE/ACT; PE accumulates alpha-weighted partials and
weight sums into persistent PSUM. Per-core partials [H, D+1] are
combined on the host (cheaper than a device AllReduce).

Tail (12500 = 24*512 + 212) uses a 1-instance-per-partition path."""

import numpy as np

from concourse import bacc, masks, mybir, tile
from concourse.bass_utils import run_bass_kernel_spmd

N, L, D, H = 100000, 16, 128, 8
NCORES = 8
NS = N // NCORES            # 12500 instances per core
CHUNK = 128                 # instances per small (tail) tile
BIG = 512                   # instances per big tile (4 per partition)
NB = 4                      # blocks per partition in a big tile
F32 = mybir.dt.float32
F16 = mybir.dt.float16
PATHS_NP_DTYPE = np.float16
AF = mybir.ActivationFunctionType

_cached_nc = None


def _build(ns=NS, repeat=1, hw_loop=1, **_compat):
    nbig = ns // BIG
    tail_chunks = []
    t0 = nbig * BIG
    rem = ns - t0
    while rem > 0:
        cnt = min(CHUNK, rem)
        tail_chunks.append((t0, cnt))
        t0 += cnt
        rem -= cnt
    nc = bacc.Bacc(
        "TRN2",
        target_bir_lowering=False,
        debug=False,
        enable_asserts=False,
        num_devices=NCORES,
    )
    paths_d = nc.dram_tensor("paths", [ns, L, D], F16, kind="ExternalInput")
    tgt_d = nc.dram_tensor("target_feat", [D], F32, kind="ExternalInput")
    af_d = nc.dram_tensor("attn_fc", [H, 2 * D], F32, kind="ExternalInput")
    out_d = nc.dram_tensor("out", [H * (D + 1)], F32, kind="ExternalOutput")

    FD = L * D  # 2048 elements per instance

    with tile.TileContext(nc) as tc:
        with (
            tc.tile_pool(name="const", bufs=1) as constp,
            tc.tile_pool(name="inp", bufs=5) as inp,
            tc.tile_pool(name="work", bufs=3) as work,
            tc.tile_pool(name="ps", bufs=1, space="PSUM") as psp,
        ):
            # ---------- constants ----------
            ident = constp.tile([128, 128], F32)
            masks.make_identity(nc, ident[:])
            ident16 = constp.tile([128, 128], F16)
            nc.vector.tensor_copy(ident16[:], ident[:])
            af = constp.tile([H, 2 * D], F32)
            nc.sync.dma_start(af[:], af_d.ap())
            tf = constp.tile([D, 1], F32)
            nc.sync.dma_start(tf[:], tgt_d.ap().rearrange("(d one) -> d one", one=1))
            ones_row = constp.tile([1, CHUNK], F32)
            nc.vector.memset(ones_row[:], 1.0)
            ones_col = constp.tile([CHUNK, 1], F16)
            nc.vector.memset(ones_col[:], 1.0)

            # a_tT [D, H]: transpose of attn_fc[:, :D] (f32, setup only)
            ps_t = psp.tile([128, 128], F32, tag="setup")
            a_tT = constp.tile([D, H], F32)
            nc.tensor.transpose(ps_t[:D, :H], af[:H, 0:D], ident[:H, :H])
            nc.vector.tensor_copy(a_tT[:], ps_t[:D, :H])
            # a_rT [D, H] scaled by 1/16 (folds the path-mean into scores),
            # cast to fp16 so it can pair with the fp16 repsum^T stationary
            ps_r = psp.tile([128, 128], F32, tag="setup")
            a_rT = constp.tile([D, H], F16)
            nc.tensor.transpose(ps_r[:D, :H], af[:H, D : 2 * D], ident[:H, :H])
            nc.scalar.mul(a_rT[:], ps_r[:D, :H], 1.0 / L)
            # per-head bias b[h] = a_t[h] . target  -> kept as a [1, H] row
            ps_b = psp.tile([128, 128], F32, tag="setup")
            b_row = constp.tile([1, H], F32)
            nc.tensor.matmul(ps_b[:1, :H], tf[:, :1], a_tT[:, :H])
            nc.vector.tensor_copy(b_row[:], ps_b[:1, :H])

            # ---------- persistent accumulators ----------
            acc_p = psp.tile([H, 4 * D], F32, tag="accP")  # 4-node partials
            acc_s = psp.tile([H, 1], F32, tag="accS")      # sum_n w[n,h]

            paths2d = paths_d.ap().rearrange("n l d -> n (l d)")

            # ---------- main streaming loop ----------
            started = [False]

            def score_block(tmp_t, off, cnt, e_dst, rT_blk):
                # repsum^T [D, cnt] from the 4 partial-sum d-blocks: fp16
                # matmuls against fp16 identity, fp32 PSUM accumulation.
                pt = psp.tile([128, CHUNK], F32, tag="pt", bufs=2)
                for j in range(4):
                    nc.tensor.matmul(
                        pt[:D, :cnt],
                        tmp_t[:cnt, off + j * D : off + (j + 1) * D],
                        ident16[:cnt, :cnt],
                        start=(j == 0), stop=(j == 3),
                        skip_group_check=True,
                    )
                nc.scalar.copy(rT_blk[:, :cnt], pt[:D, :cnt])
                nc.tensor.matmul(
                    e_dst, ones_row[:1, :cnt], b_row[:1, :],
                    start=True, stop=False, skip_group_check=True,
                )
                nc.tensor.matmul(
                    e_dst, rT_blk[:, :cnt], a_rT[:, :],
                    start=False, stop=True, skip_group_check=True,
                )

            def weights_from_scores(e_ps, wT, cnt, w):
                # LeakyReLU(0.2) then exp; wT in fp16 for the acc matmuls
                sc = work.tile([128, NB * H], F32, tag="sc")
                nc.vector.tensor_scalar_mul(sc[:cnt, :w], e_ps[:cnt, :w], 0.2)
                lr = work.tile([128, NB * H], F32, tag="lr")
                nc.vector.tensor_max(lr[:cnt, :w], sc[:cnt, :w], e_ps[:cnt, :w])
                nc.scalar.activation(wT[:cnt, :w], lr[:cnt, :w], AF.Exp)

            def do_small(n0, cnt, last):
                first = not started[0]
                started[0] = True
                t = inp.tile([128, FD], F16, tag="in", padded_shape=[128, NB * FD])
                nc.sync.dma_start(t[:cnt, :], paths2d[n0 : n0 + cnt, :])
                tmp = work.tile(
                    [128, 1024], F16, tag="tree", padded_shape=[128, NB * 1024]
                )
                nc.vector.tensor_add(
                    tmp[:cnt, :], t[:cnt, 0:1024], t[:cnt, 1024:2048]
                )
                nc.vector.tensor_add(
                    tmp[:cnt, 0:512], tmp[:cnt, 0:512], tmp[:cnt, 512:1024]
                )
                rT = work.tile([D, CHUNK], F16, tag="rT", padded_shape=[D, NB * CHUNK])
                e_ps = psp.tile(
                    [128, H], F32, tag="e", bufs=2, padded_shape=[128, NB * H]
                )
                score_block(tmp, 0, cnt, e_ps[:cnt, :], rT)
                wT = work.tile([128, H], F16, tag="wT", padded_shape=[128, NB * H])
                weights_from_scores(e_ps, wT, cnt, H)
                nc.tensor.matmul(
                    acc_p[:H, :], wT[:cnt, :H], tmp[:cnt, 0:512],
                    start=first, stop=last,
                )
                nc.tensor.matmul(
                    acc_s[:H, :], wT[:cnt, :H], ones_col[:cnt, :],
                    start=first, stop=last,
                )

            def do_big(n0, last):
                first = not started[0]
                started[0] = True
                t = inp.tile([128, NB * FD], F16, tag="in")
                t3 = t.rearrange("p (b f) -> p b f", b=NB)
                nc.sync.dma_start(
                    t3[:, :, :],
                    paths2d[n0 : n0 + BIG, :].rearrange("(b p) f -> p b f", b=NB),
                )
                tmp = work.tile([128, NB * 1024], F16, tag="tree")
                tmp3 = tmp.rearrange("p (b x) -> p b x", b=NB)
                nc.vector.tensor_add(
                    tmp3[:, :, :], t3[:, :, 0:1024], t3[:, :, 1024:2048]
                )
                nc.vector.tensor_add(
                    tmp3[:, :, 0:512], tmp3[:, :, 0:512], tmp3[:, :, 512:1024]
                )
                e_ps = psp.tile([128, NB * H], F32, tag="e", bufs=2)
                rT = work.tile([D, NB * CHUNK], F16, tag="rT")
                for b in range(NB):
                    score_block(
                        tmp,
                        b * 1024,
                        CHUNK,
                        e_ps[:, b * H : (b + 1) * H],
                        rT[:, b * CHUNK : (b + 1) * CHUNK],
                    )
                wT = work.tile([128, NB * H], F16, tag="wT")
                weights_from_scores(e_ps, wT, 128, NB * H)
                for b in range(NB):
                    bfirst = first and b == 0
                    blast = last and b == NB - 1
                    nc.tensor.matmul(
                        acc_p[:H, :],
                        wT[:, b * H : (b + 1) * H],
                        tmp[:, b * 1024 : b * 1024 + 512],
                        start=bfirst, stop=blast,
                    )
                    nc.tensor.matmul(
                        acc_s[:H, :],
                        wT[:, b * H : (b + 1) * H],
                        ones_col[:, :],
                        start=bfirst, stop=blast,
                    )

            # repeat>1 / hw_loop>1 are timing-only modes (re-stream the same
            # shard; output then over-counts, never used for correctness
            # runs). hw_loop wraps the repeat passes in a For_i hardware
            # loop so device time scales without instruction-count blowup.
            import contextlib

            loop_ctx = (
                tc.For_i(0, hw_loop) if hw_loop > 1 else contextlib.nullcontext()
            )
            with loop_ctx:
                for r in range(repeat):
                    lr_ = r == repeat - 1
                    for c in range(nbig):
                        do_big(
                            c * BIG,
                            lr_ and not tail_chunks and c == nbig - 1,
                        )
                    for i, (n0, cnt) in enumerate(tail_chunks):
                        do_small(n0, cnt, lr_ and i == len(tail_chunks) - 1)
                started[0] = False

            # ---------- emit per-core partial [p_raw | s] ----------
            # Cross-core combine + softmax normalization happens on the host
            # in kernel(): cheaper than a device AllReduce + bounce trips.
            accs = work.tile([H, 4 * D], F32, tag="accs")
            nc.vector.tensor_copy(accs[:H, :], acc_p[:H, :])
            fold = work.tile([H, 2 * D], F32, tag="fold")
            nc.vector.tensor_add(
                fold[:H, :], accs[:H, 0 : 2 * D], accs[:H, 2 * D : 4 * D]
            )
            part = work.tile([H, D + 1], F32, tag="part")
            nc.vector.tensor_add(part[:H, 0:D], fold[:H, 0:D], fold[:H, D : 2 * D])
            nc.vector.tensor_copy(part[:H, D : D + 1], acc_s[:H, :])
            nc.sync.dma_start(
                out_d.ap().rearrange("(h d) -> h d", d=D + 1), part[:]
            )

    nc.compile()
    return nc


def kernel(target_feat, paths, attn_fc, **_unused):
    global _cached_nc
    if _cached_nc is None:
        _cached_nc = _build()
    nc = _cached_nc

    paths = np.ascontiguousarray(
        np.asarray(paths, dtype=np.float32).astype(PATHS_NP_DTYPE)
    )
    shards = paths.reshape(NCORES, NS, L, D)
    tgt = np.ascontiguousarray(np.asarray(target_feat, dtype=np.float32))
    af = np.ascontiguousarray(np.asarray(attn_fc, dtype=np.float32))
    in_maps = [
        {"paths": shards[i], "target_feat": tgt, "attn_fc": af}
        for i in range(NCORES)
    ]
    res = run_bass_kernel_spmd(nc, in_maps, core_ids=list(range(NCORES)))
    # host-side combine of the 8 per-core partials [8, D+1]
    tot = np.zeros((H, D + 1), dtype=np.float64)
    for i in range(NCORES):
        tot += np.asarray(res.results[i]["out"], dtype=np.float64).reshape(
            H, D + 1
        )
    out = tot[:, :D] / (L * tot[:, D:])
    return np.ascontiguousarray(out.reshape(H * D).astype(np.float32))
